# revision 32
# baseline (speedup 1.0000x reference)
"""Multi-head linear attention ('l1' attention) Bass kernel for 8 TRN2 NeuronCores.

Reference computation (fp32, batch 4, seq n=8192, d_model=1024, 16 heads x 64):
    q = softmax(x@Wq + bq, axis=dh); k = softmax(x@Wk + bk, axis=dh); v = x@Wv + bv
    k_sum = k.sum(rows);  d_inv = 1/((q*k_sum).sum(dh))
    ctx = k^T v (per head);  y = (q @ ctx) * d_inv + q;  out = y @ Wo + bo

Algebraic form used on-chip (s2 = sum_dh exp_q, s1 = sum_dh exp_q*k_sum):
    y = (exp_q @ ctx)/s1 + exp_q/s2
so the q-softmax normalization never needs a partition-axis divide.

Sharding: sequence-parallel. Rows (b*n = 32768) split into 8 contiguous chunks
of 4096; cores {2b, 2b+1} hold batch b, and ctx/k_sum partials are summed with
a 2-core AllReduce.

Precision: K/V projections run in fp8-e4m3 DoubleRow (weights pre-scaled x256
on the host, rescaled 1/256 in the activation); Q and output projections in
bf16; softmax and normalization math in fp32/bf16.  Numpy-simulated rel-err
of this mix vs fp32 is ~5e-3 (gate is 2e-2).

Bias handling (no PE bias matmuls):
  k: exp(k+bk) = exp(k)*exp(bk); exp(bk) multiplies in on gpsimd before the
     dh-softmax sums.
  v: ctx = ksm^T(v_raw+bv) = ksm^T v_raw + ksum (x) bv: rank-1 fix applied to
     ctx once, after the AllReduce (uses the reduced ksum).
  q: folded into the exp activation (per-partition bias, feature-major).
  o: added by gpsimd during the PSUM->SBUF move.

The per-row 1/s1, 1/s2 broadcasts and the s-sums use packed concurrent
matmuls (row/col tile_position pairs) so each costs one 512-wide pass.
"""

import contextlib
import os
import sys

sys.path.insert(0, "/opt/trn_rl_repo")

import numpy as np
import ml_dtypes

import concourse.bass as bass
import concourse.mybir as mybir
import concourse.bacc as bacc
import concourse.tile as tile
from concourse.bass_utils import run_bass_kernel_spmd

BF16 = mybir.dt.bfloat16
F32 = mybir.dt.float32
F32R = mybir.dt.float32r
FP8 = mybir.dt.float8e4
EXP = mybir.ActivationFunctionType.Exp
COPY = mybir.ActivationFunctionType.Copy
MUL = mybir.AluOpType.mult
ADD = mybir.AluOpType.add
DR = mybir.MatmulPerfMode.DoubleRow

D = 1024          # d_model
NCORES = 8
BLK = 512         # rows per block (moving-operand width)
BF = ml_dtypes.bfloat16
F8 = ml_dtypes.float8_e4m3
W8SCALE = 256.0


def build_attention(tc, R):
    """Emit the kernel for one core holding R rows (R % 512 == 0)."""
    nc = tc.nc
    NB = R // BLK
    groups = [[2 * i, 2 * i + 1] for i in range(NCORES // 2)]

    # all big inputs arrive pre-tiled by the host as exact SBUF images
    # (contiguous multi-KB partition lines -> few DMA descriptors):
    #   xT/x8: [NB*128, 8*BLK], block b at rows 128b:128b+128
    #   x8q:   [4*128, 8*128], block-0 row-quarter j at rows 128j:128j+128
    #   wk8/wv8: [2*128, 4*D], contraction half i at rows 128i:128i+128
    #   wq/wo: [128, 8*D]
    xT_d = nc.dram_tensor("xT", [NB * 128, 8 * BLK], BF16,
                          kind="ExternalInput").ap()
    x8_d = nc.dram_tensor("x8", [NB * 128, 8 * BLK], FP8,
                          kind="ExternalInput").ap()
    x8q_d = nc.dram_tensor("x8q", [4 * 128, 8 * 128], FP8,
                           kind="ExternalInput").ap()
    wq_d = nc.dram_tensor("wq", [128, 8 * D], BF16, kind="ExternalInput").ap()
    wo_d = nc.dram_tensor("wo", [128, 8 * D], BF16, kind="ExternalInput").ap()
    wk8_d = nc.dram_tensor("wk8", [2 * 128, 4 * D], FP8,
                           kind="ExternalInput").ap()
    wv8_d = nc.dram_tensor("wv8", [2 * 128, 4 * D], FP8,
                           kind="ExternalInput").ap()
    bq32_d = nc.dram_tensor("bq32", [D], F32, kind="ExternalInput").ap()
    ebk_d = nc.dram_tensor("ebk_b", [128, D], BF16, kind="ExternalInput").ap()
    bv_d = nc.dram_tensor("bv_b", [128, D], BF16, kind="ExternalInput").ap()
    bo_d = nc.dram_tensor("bo_b", [128, D], BF16, kind="ExternalInput").ap()
    out_d = nc.dram_tensor("out", [R, D], BF16, kind="ExternalOutput").ap()

    with (
        tc.tile_pool(name="cpool", bufs=1) as cpool,
        tc.tile_pool(name="xpool", bufs=3) as xpool,
        tc.tile_pool(name="x8pool", bufs=2) as x8pool,
        tc.tile_pool(name="ka", bufs=3) as ka,
        tc.tile_pool(name="bp", bufs=3) as bp,
        tc.tile_pool(name="eqp", bufs=4) as eqp,
        tc.tile_pool(name="ypool", bufs=4) as ypool,
        tc.tile_pool(name="dram", bufs=1, space="DRAM") as dram,
    ):
        # ---- persistent constants ----
        # fp8 K/V weights in two half tiles each (chunk-pairs 0-1 / 2-3) so the
        # first matmuls wait only on a 512KB DMA, not the full matrix.
        # column layout of a half tile: [p, pair(2), 2*D] with the pair's two
        # 128-feature chunks adjacent -> DoubleRow 3D AP slicing is trivial.
        wk8_t = [cpool.tile([128, 4 * D], FP8, tag=f"wk8{i}", name=f"wk8{i}")
                 for i in range(2)]
        wv8_t = [cpool.tile([128, 4 * D], FP8, tag=f"wv8{i}", name=f"wv8{i}")
                 for i in range(2)]

        def load_w8(t2, d_ap, eng):
            # half i covers contraction chunks 4i..4i+3
            for i in range(2):
                eng.dma_start(t2[i][:], d_ap[128 * i : 128 * (i + 1), :])

        def w8pair(t2, c2, lo, size):
            # stationary/moving view for chunk pair c2: [128, 2, size]
            i, r = divmod(c2, 2)
            return (t2[i][:].rearrange("p (c f) -> p c f", f=D)
                    [:, 2 * r : 2 * r + 2, lo : lo + size])

        # bf16 wq/wo (loaded late; only phase B needs them)
        w_t = {n: cpool.tile([128, 8 * D], BF16, tag=f"{n}all", name=f"{n}all")
               for n in ("wq", "wo")}

        def wslice(n, c, lo, size):
            return w_t[n][:, D * c + lo : D * c + lo + size]

        def load_w(n, d_ap, eng):
            eng.dma_start(w_t[n][:], d_ap)

        ones1 = cpool.tile([1, 128], BF16, tag="ones1")
        nc.vector.memset(ones1[:], 1.0)
        ones128 = cpool.tile([128, 1], BF16, tag="ones128")
        nc.vector.memset(ones128[:], 1.0)



        def load_x8(b, eng=None):
            eng = eng or nc.sync
            t = x8pool.tile([128, 8 * BLK], FP8, tag="x8", name="x8")
            eng.dma_start(t[:], x8_d[128 * b : 128 * (b + 1), :])
            return t

        def load_x(b, eng=None):
            eng = eng or nc.sync
            t = xpool.tile([128, 8 * BLK], BF16, tag="xa", name="xa")
            eng.dma_start(t[:], xT_d[128 * b : 128 * (b + 1), :])
            return t

        # block 0 of x8 split into 4 row-quarter tiles so the first matmul
        # waits on ~128KB only
        x8q0 = [cpool.tile([128, 8 * 128], FP8, tag=f"x8q{j}", name=f"x8q{j}")
                for j in range(4)]
        for j in range(4):
            nc.sync.dma_start(x8q0[j][:], x8q_d[128 * j : 128 * (j + 1), :])
        load_w8(wk8_t, wk8_d, nc.scalar)
        load_w8(wv8_t, wv8_d, nc.gpsimd)

        # host-prebroadcast bias tiles [128, D] (behind the wv8 halves on the
        # gpsimd queue; none is needed before the first v-projection)
        ebk_b = cpool.tile([128, D], BF16, tag="ebkb")
        nc.gpsimd.dma_start(ebk_b[:], ebk_d)
        bv_b = cpool.tile([128, D], BF16, tag="bvb")
        nc.gpsimd.dma_start(bv_b[:], bv_d)
        bo_b = cpool.tile([128, D], BF16, tag="bob")
        nc.gpsimd.dma_start(bo_b[:], bo_d)
        bq_sb = cpool.tile([128, 8], F32, tag="bqsb")
        nc.gpsimd.dma_start(bq_sb[:], bq32_d.rearrange("(f p) -> p f", p=128))

        def x8pair(b, x8t, c2, j):
            # stationary view [128, 2, 128] for chunk pair c2, row quarter j
            if b == 0:
                return (x8q0[j][:].rearrange("p (c r) -> p c r", r=128)
                        [:, 2 * c2 : 2 * c2 + 2, :])
            return (x8t[:].rearrange("p (c r) -> p c r", r=BLK)
                    [:, 2 * c2 : 2 * c2 + 2, 128 * j : 128 * j + 128])

        # ================= Phase A: K/V projections, ctx & k_sum partials ====
        phaseA = contextlib.ExitStack()
        psKV = phaseA.enter_context(tc.tile_pool(name="psKV", bufs=2, space="PSUM"))
        psACC = phaseA.enter_context(tc.tile_pool(name="psACC", bufs=1, space="PSUM"))
        ctx_ps = psACC.tile([128, D], F32, tag="ctx")      # 2 banks
        ksum_ps = psACC.tile([128, 8], F32, tag="ksum")    # 1 bank
        ntiles = R // 128

        # ctx[d,e] += sum_rows ksm[r,d] * v[r,e]  (2 heads per 128-block)
        # ksum[d]  += sum_rows ksm[r,d]
        # start/stop once per PSUM bank (start marks the whole bank
        # pending-zero; later matmuls in the bank overwrite their own bytes).
        pipe = []

        def emit_ctx(t_idx, h, ksm_h, vb_h):
            first, last = t_idx == 0, t_idx == ntiles - 1
            for p4 in range(4):
                p = 4 * h + p4
                psl = slice(128 * p4, 128 * p4 + 128)
                nc.tensor.matmul(ctx_ps[:, 128 * p : 128 * p + 128],
                                 ksm_h[:, psl], vb_h[:, psl],
                                 start=(first and p4 == 0),
                                 stop=(last and p4 == 3))
                nc.tensor.matmul(ksum_ps[:, p : p + 1], ksm_h[:, psl], ones128[:],
                                 start=(first and p == 0),
                                 stop=(last and p == 7))

        xt_res = {}     # resident bf16 x tiles for phase B hoisted blocks
        for b in range(NB):
            x8t = None if b == 0 else load_x8(b)
            if b == max(0, NB - 2):
                load_w("wq", wq_d, nc.sync)
                load_w("wo", wo_d, nc.sync)
                xt_res[NB - 1] = load_x(NB - 1, nc.scalar)
            if b == NB - 1 and NB >= 2:
                xt_res[NB - 2] = load_x(NB - 2, nc.scalar)
            for j in range(4):
                t_idx = 4 * b + j
                for h in range(2):
                    hs = slice(512 * h, 512 * h + 512)
                    k_ps = psKV.tile([128, 512], F32, tag="kps", name="k_ps")
                    v_ps = psKV.tile([128, 512], F32, tag="vps", name="v_ps")
                    for c2 in range(4):
                        st = x8pair(b, x8t, c2, j)
                        nc.tensor.matmul(k_ps[:], st,
                                         w8pair(wk8_t, c2, 512 * h, 512),
                                         start=(c2 == 0), stop=(c2 == 3),
                                         perf_mode=DR)
                        nc.tensor.matmul(v_ps[:], st,
                                         w8pair(wv8_t, c2, 512 * h, 512),
                                         start=(c2 == 0), stop=(c2 == 3),
                                         perf_mode=DR)
                    # k softmax over each head's 64 columns:
                    # ke = exp(k/256); kew = ke * exp(bk); ksm = kew / seg_sum
                    ke = ka.tile([128, 512], BF16, tag="ke", name="ke")
                    nc.scalar.activation(ke[:], k_ps[:], EXP, scale=1.0 / W8SCALE)
                    kew = ka.tile([128, 512], BF16, tag="kew", name="kew")
                    nc.gpsimd.tensor_tensor(kew[:], ke[:], ebk_b[:, hs], op=MUL)
                    ks = ka.tile([128, 8], F32, tag="ks", name="ks")
                    nc.vector.reduce_sum(ks[:],
                                         kew[:].rearrange("p (n s) -> p n s", s=64),
                                         axis=mybir.AxisListType.X)
                    kr = ka.tile([128, 8], F32, tag="kr", name="kr")
                    nc.vector.reciprocal(kr[:], ks[:])
                    ksm_h = ka.tile([128, 512], BF16, tag="ksm", name="ksm_h")
                    nc.vector.tensor_tensor(
                        ksm_h[:].rearrange("p (n s) -> p n s", s=64),
                        kew[:].rearrange("p (n s) -> p n s", s=64),
                        kr[:].unsqueeze(2).broadcast_to([128, 8, 64]),
                        op=MUL,
                    )
                    vb_h = ka.tile([128, 512], BF16, tag="vb", name="vb_h")
                    nc.scalar.activation(vb_h[:], v_ps[:], COPY,
                                         scale=1.0 / W8SCALE)
                    # ctx/ksum matmuls run a half-tile behind the projections
                    # so the PE never waits on the current softmax chain.
                    pipe.append((t_idx, h, ksm_h, vb_h))
                    if len(pipe) > 2:
                        emit_ctx(*pipe.pop(0))

        while pipe:
            emit_ctx(*pipe.pop(0))

        # Pack only the useful diagonal 64x64 blocks of each head-pair ctx
        # block (plus ksum) into one compact buffer for the AllReduce.
        pack_sb = cpool.tile([128, 520], F32, tag="packsb")
        for p in range(8):
            nc.vector.tensor_copy(pack_sb[0:64, 64 * p : 64 * p + 64],
                                  ctx_ps[0:64, 128 * p : 128 * p + 64])
            nc.vector.tensor_copy(pack_sb[64:128, 64 * p : 64 * p + 64],
                                  ctx_ps[64:128, 128 * p + 64 : 128 * p + 128])
        nc.vector.tensor_copy(pack_sb[:, 512:520], ksum_ps[:])

        phaseA.close()

        ctx_bf = ksel = sel = None

        def emit_collective():
            nonlocal ctx_bf, ksel, sel
            # ====== AllReduce ctx & k_sum across the 2 cores holding each batch
            bounce_in = dram.tile([128, 520], F32)
            bounce_out = dram.tile([128, 520], F32)
            nc.sync.dma_start(bounce_in[:], pack_sb[:])
            nc.gpsimd.collective_compute(
                "AllReduce",
                mybir.AluOpType.add,
                replica_groups=groups,
                ins=[bounce_in.opt()],
                outs=[bounce_out.opt()],
            )
            unpack_sb = cpool.tile([128, 520], F32, tag="unpacksb")
            nc.sync.dma_start(unpack_sb[:], bounce_out[:])
            # rebuild block-diagonal bf16 ctx (off-diagonal zero), folding in
            # the rank-1 v-bias fix: ctx += ksum (x) bv  (per head)
            ctx_bf = cpool.tile([128, D], BF16, tag="ctxbf")
            nc.vector.memset(ctx_bf[:], 0.0)
            for p in range(8):
                nc.vector.scalar_tensor_tensor(
                    ctx_bf[0:64, 128 * p : 128 * p + 64],
                    bv_b[0:64, 128 * p : 128 * p + 64],
                    unpack_sb[0:64, 512 + p : 513 + p],
                    unpack_sb[0:64, 64 * p : 64 * p + 64],
                    op0=MUL, op1=ADD,
                )
                nc.vector.scalar_tensor_tensor(
                    ctx_bf[64:128, 128 * p + 64 : 128 * p + 128],
                    bv_b[64:128, 128 * p + 64 : 128 * p + 128],
                    unpack_sb[64:128, 512 + p : 513 + p],
                    unpack_sb[64:128, 64 * p : 64 * p + 64],
                    op0=MUL, op1=ADD,
                )

            # ksel[:, 4f:4f+4] = [ksum_h1 | ksum_h2 | 1_h1 | 1_h2] for chunk f
            ksel = cpool.tile([128, 32], BF16, tag="ksel")
            nc.vector.memset(ksel[:], 0.0)
            for f in range(8):
                nc.vector.tensor_copy(ksel[0:64, 4 * f : 4 * f + 1],
                                      unpack_sb[0:64, 512 + f : 513 + f])
                nc.vector.tensor_copy(ksel[64:128, 4 * f + 1 : 4 * f + 2],
                                      unpack_sb[64:128, 512 + f : 513 + f])
                nc.vector.memset(ksel[0:64, 4 * f + 2 : 4 * f + 3], 1.0)
                nc.vector.memset(ksel[64:128, 4 * f + 3 : 4 * f + 4], 1.0)

            # head-block broadcast selectors: A from rows 0:2 (1/s1), B rows 2:4 (1/s2)
            sel_np = np.zeros((4, 256), ml_dtypes.bfloat16)
            sel_np[0, 0:64] = 1.0
            sel_np[1, 64:128] = 1.0
            sel_np[2, 128:192] = 1.0
            sel_np[3, 192:256] = 1.0
            sel_dram = nc.inline_tensor(sel_np, name="selconst")
            sel = cpool.tile([4, 256], BF16, tag="sel")
            nc.gpsimd.dma_start(sel[:], sel_dram.ap())

        # ================= Phase B: Q path, y, output projection ==============
        # qproj_exp(b) has no dependency on the AllReduce, so those matmuls
        # overlap the collective; finish(b) consumes ctx/ksum.
        from concourse.dve_ops import RECIP_APPROX_FAST_CONSTS, RECIPROCAL_APPROX_FAST

        phaseB = contextlib.ExitStack()
        psQT = phaseB.enter_context(tc.tile_pool(name="psQT", bufs=2, space="PSUM"))
        psOPS = phaseB.enter_context(tc.tile_pool(name="psOPS", bufs=2, space="PSUM"))
        psY1 = phaseB.enter_context(tc.tile_pool(name="psY1", bufs=1, space="PSUM"))
        psS = phaseB.enter_context(tc.tile_pool(name="psS", bufs=1, space="PSUM"))
        psAB = phaseB.enter_context(tc.tile_pool(name="psAB", bufs=1, space="PSUM"))

        def qproj_f(b, xt, f):
            qT_ps = psQT.tile([128, BLK], F32, tag="qT", name="qT_ps")
            for c in range(8):
                nc.tensor.matmul(qT_ps[:], wslice("wq", c, 128 * f, 128),
                                 xt[:, BLK * c : BLK * c + BLK],
                                 start=(c == 0), stop=(c == 7))
            eq = eqp.tile([128, BLK], BF16, tag=f"eq{f}", name="eq")
            nc.scalar.activation(eq[:], qT_ps[:], EXP, bias=bq_sb[:, f : f + 1])
            return eq

        def qproj_exp(b, xt):
            return [qproj_f(b, xt, f) for f in range(8)]

        def finish_f(b, eqs, f, yT, filler=None):
            fs = slice(128 * f, 128 * f + 128)
            eq = eqs[f]
            s_ps = psS.tile([4, BLK], F32, tag="s", name="s_ps")
            nc.tensor.matmul(s_ps[:], ksel[:, 4 * f : 4 * f + 4], eq[:],
                             start=True, stop=True)
            y1_ps = psY1.tile([128, BLK], F32, tag="y1", name="y1_ps")
            nc.tensor.matmul(y1_ps[:], ctx_bf[:, fs], eq[:], start=True, stop=True)
            rs = bp.tile([4, BLK], BF16, tag="rs", name="rs")
            cst = RECIP_APPROX_FAST_CONSTS
            with nc.allow_low_precision(reason="f32r feed for broadcast matmul"):
                nc.vector._custom_dve(RECIPROCAL_APPROX_FAST, out=rs[:],
                                      in0=s_ps[:], s0=cst["s0"], s1=cst["s1"],
                                      imm2=cst["imm2"])
            if filler is not None:
                filler()      # dense PE work to cover the recip round-trip
            A_ps = psAB.tile([128, BLK], F32, tag="Ab", name="A_ps")
            nc.tensor.matmul(A_ps[:], sel[:, 0:128], rs[:], start=True, stop=True)
            B_ps = psAB.tile([128, BLK], F32, tag="Bb", name="B_ps")
            nc.tensor.matmul(B_ps[:], sel[:, 128:256], rs[:], start=True, stop=True)
            y1_sb = bp.tile([128, BLK], F32, tag="y1s", name="y1_sb")
            nc.scalar.copy(y1_sb[:], y1_ps[:])
            t1 = bp.tile([128, BLK], BF16, tag="t1", name="t1")
            nc.vector.tensor_tensor(t1[:], y1_sb[:], A_ps[:], op=MUL)
            t2 = bp.tile([128, BLK], BF16, tag="t2", name="t2")
            nc.vector.tensor_tensor(t2[:], eq[:], B_ps[:], op=MUL)
            yt = ypool.tile([128, BLK], BF16, tag=f"yT{f}", name="yt")
            nc.vector.tensor_tensor(yt[:], t1[:], t2[:], op=ADD)
            yT.append(yt)

        out_qs = [nc.sync, nc.scalar, nc.gpsimd]
        osb_cur = {}

        def out_group(b, yT, h, j):
            # h=0/h=1 share one [128, 1024] tile; the DMA fires once per
            # (b, j) pair and writes full 2KB DRAM rows (half the
            # descriptors of per-half writes)
            hs = slice(512 * h, 512 * h + 512)
            o_ps = psOPS.tile([128, BLK], F32, tag="ops", name="o_ps")
            for c in range(8):
                nc.tensor.matmul(o_ps[:], yT[c][:, 128 * j : 128 * j + 128],
                                 wslice("wo", c, 512 * h, 512),
                                 start=(c == 0), stop=(c == 7))
            if h == 0:
                osb_cur[(b, j)] = bp.tile([128, D], BF16, tag="osb",
                                          name="o_sb")
            o_sb = osb_cur[(b, j)]
            nc.vector.tensor_tensor(o_sb[:, hs], o_ps[:], bo_b[:, hs], op=ADD)
            if h == 1:
                r0 = BLK * b + 128 * j
                out_qs[(4 * b + j) % 3].dma_start(out_d[r0 : r0 + 128, :],
                                                  osb_cur.pop((b, j))[:])

        def finish_out(b, yT):
            for j in range(4):
                for h in range(2):
                    out_group(b, yT, h, j)

        # Schedule: blocks processed in REVERSE order so the two hoisted
        # q-projections reuse the bf16 x tiles prefetched in phase A's tail
        # (no DMA wait under the collective).  q-projection f-steps of the
        # current block interleave with the finish f-steps of the block
        # `hoist` earlier; the last finished blocks' output projections are
        # deferred to serve as PE filler under the tail finish chains.
        order = list(range(NB - 1, -1, -1))
        hoist = min(3, NB)
        eqs_map = {}
        for i in range(hoist):
            b = order[i]
            xt = xt_res.get(b)
            if xt is None:
                xt = load_x(b)
            eqs_map[b] = qproj_exp(b, xt)
        emit_collective()
        xts = {}
        if hoist < NB:
            xts[order[hoist]] = load_x(order[hoist])
        yTd = {}
        deferred = []
        for i in range(hoist, NB):
            b = order[i]
            if i + 1 < NB:
                xts[order[i + 1]] = load_x(order[i + 1])
            xt = xts.pop(b)
            fb = order[i - hoist]
            eqs_map[b] = []
            yTd[fb] = []
            eqs_map[b].append(qproj_f(b, xt, 0))
            for f in range(8):
                qfill = None
                if f < 7:
                    def qfill(b=b, xt=xt, f=f):
                        eqs_map[b].append(qproj_f(b, xt, f + 1))
                finish_f(fb, eqs_map[fb], f, yTd[fb], filler=qfill)
            if i - hoist < NB - hoist - 2:
                finish_out(fb, yTd[fb])
            else:
                deferred.append(fb)
        rem = [order[i] for i in range(max(0, NB - hoist), NB)]
        og = [(b, h, j) for b in deferred for j in range(4) for h in range(2)]
        gi = [0]

        def filler():
            if gi[0] < len(og):
                gb, h, j = og[gi[0]]
                gi[0] += 1
                out_group(gb, yTd[gb], h, j)

        # Tail: fb-major so each rem block's output projections join the
        # filler list as soon as that block is finished -> fillers stay
        # available and the out-DMAs spread instead of bursting at the end.
        for fb in rem:
            yTd[fb] = []
        for fb in rem:
            for f in range(8):
                finish_f(fb, eqs_map[fb], f, yTd[fb], filler=filler)
            og.extend((fb, h, j) for j in range(4) for h in range(2))
        while gi[0] < len(og):
            filler()
        phaseB.close()


_NC_CACHE = {}


def build_nc(R):
    if R in _NC_CACHE:
        return _NC_CACHE[R]
    nc = bacc.Bacc("TRN2", target_bir_lowering=False, debug=False,
                   num_devices=NCORES)
    with tile.TileContext(nc) as tc:
        build_attention(tc, R)
    nc.compile()
    _NC_CACHE[R] = nc
    return nc


def _tile_x(core_rows, NB):
    """[R, D] row-major -> [NB*128, 8*BLK] SBUF image (block-major)."""
    # [NB, BLK, 8, 128] -> [NB, 128(p), 8(c), BLK(r)]
    t = core_rows.reshape(NB, BLK, 8, 128).transpose(0, 3, 2, 1)
    return np.ascontiguousarray(t).reshape(NB * 128, 8 * BLK)


def _tile_w(W):
    """[D, D] -> [128, 8*D] SBUF image (chunk-major columns)."""
    t = W.reshape(8, 128, D).transpose(1, 0, 2)
    return np.ascontiguousarray(t).reshape(128, 8 * D)


def _tile_w8(W8):
    """[D, D] fp8 -> [2*128, 4*D] (contraction half-major)."""
    t = W8.reshape(2, 4, 128, D).transpose(0, 2, 1, 3)
    return np.ascontiguousarray(t).reshape(2 * 128, 4 * D)


def make_in_maps(x, Wq, bq, Wk, bk, Wv, bv, Wo, bo):
    """Host-side prep: cast/scale, pre-tile to SBUF images, shard rows."""
    b, n, d = x.shape
    assert d == D
    flat = np.asarray(x, dtype=np.float32).reshape(-1, d)
    R = flat.shape[0] // NCORES
    NB = R // BLK
    bcast = lambda v: np.ascontiguousarray(
        np.broadcast_to(np.asarray(v, np.float32).astype(BF), (128, D)))
    shared = {
        "wq": _tile_w(np.asarray(Wq, np.float32).astype(BF)),
        "wo": _tile_w(np.asarray(Wo, np.float32).astype(BF)),
        "wk8": _tile_w8((np.asarray(Wk, np.float32) * W8SCALE).astype(F8)),
        "wv8": _tile_w8((np.asarray(Wv, np.float32) * W8SCALE).astype(F8)),
        "bq32": np.asarray(bq, np.float32),
        "ebk_b": bcast(np.exp(np.asarray(bk, np.float64)).astype(np.float32)),
        "bv_b": bcast(bv),
        "bo_b": bcast(bo),
    }
    in_maps = []
    for c in range(NCORES):
        rows = flat[c * R : (c + 1) * R]
        xt = _tile_x(rows.astype(BF), NB)
        x8t = _tile_x(rows.astype(F8), NB)
        # block-0 row-quarter tiles: [4*128, 8*128]
        q = rows[:BLK].astype(F8).reshape(4, 128, 8, 128).transpose(0, 3, 2, 1)
        x8q = np.ascontiguousarray(q).reshape(4 * 128, 8 * 128)
        in_maps.append({"xT": xt, "x8": x8t, "x8q": x8q, **shared})
    return in_maps, R


def kernel(x, Wq, bq, Wk, bk, Wv, bv, Wo, bo, trace=False, **extra_kwargs):
    b, n, d = x.shape
    in_maps, R = make_in_maps(x, Wq, bq, Wk, bk, Wv, bv, Wo, bo)
    assert n % R == 0 or R % n == 0
    nc = build_nc(R)
    res = run_bass_kernel_spmd(nc, in_maps, core_ids=list(range(NCORES)),
                               trace=trace)
    out = np.concatenate([res.results[c]["out"].astype(np.float32)
                          for c in range(NCORES)], axis=0)
    out = out.reshape(b, n, d)
    if trace:
        return out, res
    return out
